# revision 17
# baseline (speedup 1.0000x reference)
"""BitLevelMapper forward (CUMULATIVE context + FLIP output) on 8 trn2 NeuronCores.

v6 = v5 + per-sub-gather fix/out slices for finer pipeline overlap
(v5 = v3 with the fold matmuls accumulating into one wide PSUM tile
(single contiguous PSUM->SBUF copy) and the idx tree reading the int32
input directly (no f32 cast copy):
  - 6-op DVE shift-add tree replaces cast+mult+reduce for the LUT row index
    R = sum_j wt(j) b_j  (wt: 2^(8-j) for j in [1,8], 2^(23-j) for j in [9,15])
  - the 128->16 partition fold keeps the 8 selection matmuls but the
    PSUM->SBUF copies are now CONTIGUOUS (dest (a c) layout); a single
    strided (a c)->(c a) interleave copy on the ACT engine produces the
    wrapped int16 index tensor the dma_gather ucode wants.
  - b15 extraction and the interleave run on the Activation engine, the out
    DMA on ACT HWDGE, input on SP HWDGE, matmuls on PE, gathers on gpsimd —
    so the DVE carries only ~10K cycles/chunk.

dma_gather consumes indices int16 "wrapped in 16 partitions" (stream
position i at partition i%16, free i//16, replicated across 16-partition
groups) and writes gathered element i to partition i%128, free i//128.
"""

import sys

sys.path.insert(0, "/opt/trn_rl_repo")

import numpy as np

from concourse import bacc, bass, mybir, tile

F32 = mybir.dt.float32
I32 = mybir.dt.int32
I16 = mybir.dt.int16
P = 128
NB = 16
TAB = 1 << 15          # table columns / LUT rows
PITCH = 64             # LUT row pitch in f32 elements (256B, dma_gather req)
BATCH = 4194304
N_CORES = 8
B_CORE = BATCH // N_CORES      # 524288 rows per core
ROWS_CHUNK = 32768             # rows per pipeline chunk
C = ROWS_CHUNK // P            # 256 rows per partition per chunk
AX = mybir.AxisListType
OP = mybir.AluOpType
AF = mybir.ActivationFunctionType


def emit_dma_gather(nc, out_ap, in_ap, idxs_ap, num_idxs, elem_size, elem_step, queue_num=0):
    """nc.gpsimd.dma_gather minus the 256B elem_size assert (the ucode only
    needs the row *stride* to be a 256B multiple; the payload can be 64B)."""
    g = nc.gpsimd
    stride_bytes = elem_step * mybir.dt.size(in_ap.dtype)
    assert stride_bytes % 256 == 0 and stride_bytes // 256 < 256
    _in_ap = g.lower_ap_dma(in_ap, for_custom_bir_dma=True)
    _idxs_ap = g.lower_ap(idxs_ap)
    _out_ap = g.lower_ap(out_ap)
    return g.add_instruction(
        mybir.InstDMAGatherAnt(
            name=nc.get_next_instruction_name(),
            ins=[*_in_ap, _idxs_ap, g.lower_val_access(g.to_reg(num_idxs))],
            outs=[_out_ap],
            transpose=False,
            num_idxs=num_idxs,
            elem_size=elem_size,
            stride_bytes_256=stride_bytes // 256,
            gen_mode=0,
            single_packet=(num_idxs <= 1024),
            queue_num=queue_num,
            sbuf_tokens_per_rank=0,
            sbuf_free_dim_per_rank=0,
            sbuf_free_dim_pad_per_rank=0,
            sbuf_byte_offset=0,
        )
    )


def build_module(b_core=B_CORE, repeat=1, ablate=(), NSUB_CFG=8, timing=False,
                 loop_all=False, n_queues=4, act_interleave=True):
    chunks = b_core // ROWS_CHUNK
    assert chunks * ROWS_CHUNK == b_core

    nc = bacc.Bacc("TRN2", target_bir_lowering=False, debug=False, num_devices=N_CORES,
                   dynamic_dma_scratch_size=32768, num_swdge_queues=n_queues)
    if timing:
        bits = nc.dram_tensor("bits", [ROWS_CHUNK, NB], I32, kind="ExternalInput")
        out = nc.dram_tensor("out", [P, NB], F32, kind="ExternalOutput")
    else:
        bits = nc.dram_tensor("bits", [b_core, NB], I32, kind="ExternalInput")
        out = nc.dram_tensor("out", [b_core, NB], F32, kind="ExternalOutput")
    tables = nc.dram_tensor("tables", [NB, TAB], F32, kind="ExternalInput")

    with tile.TileContext(nc) as tc:
        with (
            tc.tile_pool(name="const", bufs=1) as constp,
            tc.tile_pool(name="bsrc", bufs=3) as bsrcp,
            tc.tile_pool(name="psum", bufs=2, space="PSUM") as psump,
            tc.tile_pool(name="psumg", bufs=2, space="PSUM") as psumgp,
            tc.tile_pool(name="dram", bufs=1, space="DRAM") as dramp,
            tc.tile_pool(name="mbuf", bufs=1) as mp,
            tc.tile_pool(name="big", bufs=2) as bigp,
            tc.tile_pool(name="small", bufs=2) as smallp,
            tc.tile_pool(name="tree", bufs=2) as treep,
            tc.tile_pool(name="gbuf", bufs=3) as gp,
        ):
            rep_all_ctx = tc.For_i(0, repeat, 1) if (repeat > 1 and loop_all) else None
            if rep_all_ctx is not None:
                rep_all_ctx.__enter__()
            # ---------------- one-time LUT build ----------------
            J = constp.tile([NB, NB], F32)
            nc.gpsimd.memset(J[:], 0.0)
            nc.gpsimd.affine_select(
                out=J[:], in_=J[:], compare_op=OP.not_equal,
                fill=1.0, base=-(NB - 1), pattern=[[1, NB]], channel_multiplier=1,
            )

            # M[p, c*16+j] will become LUT row a = c*128+p, column j.
            M = mp.tile([P, C * NB], F32)
            for cb in range(TAB // 1024):          # 32 source tiles [16, 1024]
                src = bsrcp.tile([NB, 1024], F32, tag="src")
                nc.sync.dma_start(out=src[:], in_=tables[:, cb * 1024:(cb + 1) * 1024])
                ps = psump.tile([P, P], F32, tag="ps")
                for t in range(8):
                    nc.tensor.transpose(
                        out=ps[:, t * NB:(t + 1) * NB],
                        in_=src[:, t * P:(t + 1) * P],
                        identity=J[:],
                    )
                nc.vector.tensor_copy(out=M[:, cb * P:(cb + 1) * P], in_=ps[:])

            M3 = M[:].rearrange("p (c j) -> p c j", j=NB)
            for m in range(1, 8):
                nc.vector.tensor_copy(
                    out=M3[:, 1 << m:1 << (m + 1), 8 - m:8],
                    in_=M3[:, 0:1 << m, 8 - m:8],
                )

            TBt = constp.tile([NB, P], F32)
            nc.sync.dma_start(out=TBt[:], in_=tables[:, 0:P])
            for n in range(0, 7):
                nc.vector.tensor_copy(
                    out=TBt[0:n + 1, 1 << n:1 << (n + 1)], in_=TBt[0:n + 1, 0:1 << n]
                )
            Ft = constp.tile([NB, P], F32)
            nc.vector.memset(Ft[:], 0.0)
            for n in range(0, 7):
                nc.vector.tensor_scalar(
                    out=Ft[0:n + 1, 1 << n:1 << (n + 1)],
                    in0=Ft[0:n + 1, 0:1 << n],
                    scalar1=J[0:n + 1, 15 - n:16 - n], scalar2=None,
                    op0=OP.add,
                )
            ps2 = psump.tile([P, P], F32, tag="ps")
            nc.tensor.transpose(out=ps2[:, 0:NB], in_=TBt[:], identity=J[:])
            nc.tensor.transpose(out=ps2[:, NB:2 * NB], in_=Ft[:], identity=J[:])
            SB = constp.tile([P, 2 * NB], F32)
            nc.vector.tensor_copy(out=SB[:], in_=ps2[:, 0:2 * NB])
            Sv, Bv = SB[:, 0:NB], SB[:, NB:2 * NB]
            SP = constp.tile([P, NB], F32)   # S' = S xor B = S + B - 2SB
            t1 = constp.tile([P, NB], F32)
            nc.vector.tensor_mul(out=t1[:], in0=Sv, in1=Bv)
            nc.vector.tensor_add(out=SP[:], in0=Sv, in1=Bv)
            nc.vector.scalar_tensor_tensor(
                out=SP[:], in0=t1[:], scalar=-2.0, in1=SP[:],
                op0=OP.mult, op1=OP.add,
            )
            for j in range(8, NB):
                nc.vector.tensor_scalar(
                    out=M3[:, :, j], in0=M3[:, :, j],
                    scalar1=0.0, scalar2=SP[:, j:j + 1],
                    op0=OP.mult, op1=OP.add,
                )

            for j in range(1, 9):
                m = 8 - j  # bit m of c
                v = M[:].rearrange(
                    "p (co par ci j) -> p co par ci j", par=2, ci=1 << m, j=NB
                )[:, :, 1, :, j]
                nc.vector.tensor_scalar(
                    out=v, in0=v, scalar1=-1.0, scalar2=1.0,
                    op0=OP.mult, op1=OP.add,
                )

            # store to DRAM at 256B row pitch; row r = p*256 + c
            lut4 = dramp.tile([TAB, PITCH], F32)
            nc.sync.dma_start(
                out=lut4[:, 0:NB].rearrange("(p c) j -> p c j", p=P),
                in_=M3,
            )

            # selection matrices for the wrapped-index fold:
            # S[p, a*128+q] = 1 iff p == 16a + (q mod 16)
            Smat = constp.tile([P, 8 * P], F32)
            nc.gpsimd.memset(Smat[:], 0.0)
            nc.gpsimd.affine_select(
                out=Smat[:], in_=Smat[:], compare_op=OP.not_equal,
                fill=1.0, base=0,
                pattern=[[-16, 8], [0, 8], [-1, 16]], channel_multiplier=1,
            )

            # ---------------- main loop ----------------
            bits_v = bits[:].rearrange("(ch p c) j -> ch p (c j)", p=P, c=C)
            if timing:
                out_scr = dramp.tile([2 * ROWS_CHUNK, NB], F32)
                out_v = out_scr[:].rearrange("(ch p c) j -> ch p (c j)", p=P, c=C)
            else:
                out_v = out[:].rearrange("(ch p c) j -> ch p (c j)", p=P, c=C)
            rep_ctx = (
                tc.For_i(0, repeat, 1) if (repeat > 1 and not loop_all) else None
            )
            if rep_ctx is not None:
                rep_ctx.__enter__()
            for ch in range(chunks):
                bt = bigp.tile([P, C * NB], I32, tag="bt")
                if "in" not in ablate:
                    nc.sync.dma_start(out=bt[:], in_=bits_v[0 if timing else ch])
                bt3 = bt[:].rearrange("p (c j) -> p c j", j=NB)
                bf3 = bt3  # tree reads int32 directly; DVE converts on read
                b15 = smallp.tile([P, C], F32, tag="b15")
                idxf = smallp.tile([P, C], F32, tag="idxf")
                if "idx" not in ablate:
                    nc.scalar.activation(out=b15[:], in_=bf3[:, :, 0], func=AF.Copy)
                    # tree: s_j = b_j + 128*b_{j+8} (j=1..7)
                    s = treep.tile([P, C * 7], F32, tag="s")
                    s3 = s[:].rearrange("p (c m) -> p c m", m=7)
                    nc.vector.scalar_tensor_tensor(
                        out=s3, in0=bf3[:, :, 9:16], scalar=128.0,
                        in1=bf3[:, :, 1:8], op0=OP.mult, op1=OP.add,
                    )
                    t = treep.tile([P, C * 4], F32, tag="t")
                    t3 = t[:].rearrange("p (c m) -> p c m", m=4)
                    nc.vector.scalar_tensor_tensor(
                        out=t3[:, :, 0:3], in0=s3[:, :, 0:6:2], scalar=2.0,
                        in1=s3[:, :, 1:7:2], op0=OP.mult, op1=OP.add,
                    )
                    nc.vector.scalar_tensor_tensor(
                        out=t3[:, :, 3], in0=s3[:, :, 6], scalar=2.0,
                        in1=bf3[:, :, 8], op0=OP.mult, op1=OP.add,
                    )
                    u = treep.tile([P, C * 2], F32, tag="u2")
                    u3 = u[:].rearrange("p (c m) -> p c m", m=2)
                    nc.vector.scalar_tensor_tensor(
                        out=u3, in0=t3[:, :, 0:4:2], scalar=4.0,
                        in1=t3[:, :, 1:4:2], op0=OP.mult, op1=OP.add,
                    )
                    nc.vector.scalar_tensor_tensor(
                        out=idxf[:], in0=u3[:, :, 0], scalar=16.0,
                        in1=u3[:, :, 1], op0=OP.mult, op1=OP.add,
                    )
                else:
                    nc.vector.memset(idxf[:], 1.0)
                    nc.vector.memset(b15[:], 0.0)

                # fold idxf[128, C] -> wrapped wr[q(16), c*8+a] = idxf[16a+q, c]
                wr = bigp.tile([P, 8 * C], I16, tag="wr")
                if "fold" not in ablate:
                    wrT = bigp.tile([P, 8 * C], I16, tag="wrT")
                    for h in range(2):
                        pg = psumgp.tile([P, 4 * C], F32, tag="pg")
                        for a4 in range(4):
                            a = 4 * h + a4
                            nc.tensor.matmul(
                                out=pg[:, a4 * C:(a4 + 1) * C],
                                lhsT=Smat[:, a * P:(a + 1) * P], rhs=idxf[:],
                                start=True, stop=True,
                            )
                        nc.vector.tensor_copy(
                            out=wrT[:, h * 4 * C:(h + 1) * 4 * C], in_=pg[:]
                        )
                    wrTv = wrT[:].rearrange("p (a c) -> p c a", a=8)
                    wr3 = wr[:].rearrange("p (c a) -> p c a", a=8)
                    if act_interleave:
                        nc.scalar.activation(out=wr3, in_=wrTv, func=AF.Copy)
                    else:
                        nc.vector.tensor_copy(out=wr3, in_=wrTv)
                else:
                    nc.vector.memset(wr[:], 1)

                G = gp.tile([P, C * NB], F32, tag="G")
                G3 = G[:].rearrange("p (c j) -> p c j", j=NB)
                NSUB = NSUB_CFG
                SUBI = ROWS_CHUNK // NSUB
                SUBC = SUBI // P
                for g in range(NSUB if "gather" not in ablate else 0):
                    emit_dma_gather(
                        nc,
                        out_ap=G3[:, g * SUBC:(g + 1) * SUBC, :],
                        in_ap=lut4[:, 0:NB],
                        idxs_ap=wr[:, g * (SUBI // 16):(g + 1) * (SUBI // 16)],
                        num_idxs=SUBI,
                        elem_size=NB,
                        elem_step=PITCH,
                        queue_num=g % n_queues,
                    )

                # col 0 (bit 15): g ^= b15  ->  g*(1-2b) + b, then store —
                # both sliced per sub-gather so each slice flows as soon as
                # its gather lands.
                uf = smallp.tile([P, C], F32, tag="uf")
                if "fix" not in ablate:
                    nc.vector.tensor_scalar(
                        out=uf[:], in0=b15[:], scalar1=-2.0, scalar2=1.0,
                        op0=OP.mult, op1=OP.add,
                    )
                t2 = smallp.tile([P, C], F32, tag="t2")
                ov = out_v[ch % 2 if timing else ch].rearrange(
                    "p (c j) -> p c j", j=NB
                )
                for g in range(NSUB):
                    sl = slice(g * SUBC, (g + 1) * SUBC)
                    if "fix" not in ablate:
                        G0 = G3[:, sl, 0]
                        nc.vector.tensor_mul(out=t2[:, sl], in0=G0, in1=uf[:, sl])
                        nc.vector.tensor_add(out=G0, in0=t2[:, sl], in1=b15[:, sl])
                    if "out" not in ablate:
                        nc.scalar.dma_start(out=ov[:, sl, :], in_=G3[:, sl, :])

            if rep_ctx is not None:
                rep_ctx.__exit__(None, None, None)
            if rep_all_ctx is not None:
                rep_all_ctx.__exit__(None, None, None)
            if timing:
                nc.sync.dma_start(out=out[:], in_=SP[:])

    nc.compile()
    return nc


_NC_CACHE = {}


def _get_module(b_core, repeat=1, ablate=(), timing=False, loop_all=False):
    key = (b_core, repeat, tuple(ablate), timing, loop_all)
    if key not in _NC_CACHE:
        _NC_CACHE[key] = build_module(
            b_core, repeat, ablate=ablate, timing=timing, loop_all=loop_all
        )
    return _NC_CACHE[key]


def kernel(bits: np.ndarray, tables: np.ndarray) -> np.ndarray:
    from concourse.bass_utils import run_bass_kernel_spmd

    bits = np.ascontiguousarray(np.asarray(bits, dtype=np.int32))
    tables = np.ascontiguousarray(np.asarray(tables, dtype=np.float32))
    assert bits.shape == (BATCH, NB) and tables.shape == (NB, TAB)

    nc = _get_module(B_CORE)
    shards = np.split(bits, N_CORES, axis=0)
    in_maps = [{"bits": s, "tables": tables} for s in shards]
    res = run_bass_kernel_spmd(nc, in_maps, list(range(N_CORES)))
    return np.concatenate([r["out"] for r in res.results], axis=0)
